# revision 1
# baseline (speedup 1.0000x reference)
"""Trainium2 Bass kernel for nn_DimeNetOutput (gnn message passing).

Computes, for E=1M edges / N=100K nodes / D=64:
    x        = (edge_attr @ We + be) * msg_emb          # [E, 64]
    node_emb = segment_sum(x, edge_dst, N)              # [N, 64]
    h        = relu(node_emb @ W1 + b1)  (applied 3x, same weights)
    out      = h @ W4                                   # [N, 64]

Strategy (8 NeuronCores, node-range sharding -> no collectives):
  * Host: sort edges by dst; core c owns nodes [c*12500, (c+1)*12500).
    Each node's edge list is split into two aligned halves (A/B) so one SBUF
    column = 2 edges x 64 features = 128 partitions.  Groups of 1250 nodes are
    padded to a fixed 7168 columns.  edge_attr is packed transposed (2-block,
    fp16) so the PE streams it against a block-diag(We, We) stationary; msg_emb
    is packed transposed+stacked (fp16).
  * Device per core: PE matmul -> DVE fused (xlin+be)*msg -> DVE f32 prefix
    scan along columns -> GPSIMD ap_gather of the per-node end columns ->
    DVE diff (exact f32 segment sums) -> 4 matmul MLP (PE) with relu+bias on
    ACT -> output [128, 6250] packed, unpacked on host.
"""

import os

import numpy as np

# ----------------------------------------------------------------- constants
E = 1_000_000
N = 100_000
A_DIM = 16
D = 64
NCORES = 8

NPC = N // NCORES          # 12500 nodes per core
G_NODES = 1250             # nodes per gather group
NG = NPC // G_NODES        # 10 groups per core
C = 7168                   # padded columns per group (multiple of 512)
NCHUNK = C // 512          # 14 matmul chunks of 512 per group
CT = NG * C                # 71680 columns per core
ZERO_COL = C               # reserved all-zero column index in scan window
GSLOT = 1280               # gather slots per group (1 + 1250 valid, pad to 16x;
                           # even GS16 keeps per-group idx slices 4B-aligned)
GS16 = GSLOT // 16         # 80
NBANDS = (NCHUNK + 3) // 4  # 4 column bands in the packed attr layout
ACOLS = NBANDS * 512       # 2048 packed attr cols per group
MLP_C1 = 250               # layer-1 chunk (50 chunks)
HCOLS = NPC // 2           # 6250 stacked columns for layers 2..4
MLP_C2 = 500               # layer-2..4 chunk

_F16 = np.float16
_PROG = {}


# ------------------------------------------------------------- host packing
def pack_inputs(edge_attr, msg_emb, edge_dst, We, be, W1, b1, W4):
    """Build the 8 per-core input maps (numpy only)."""
    dst = np.asarray(edge_dst).astype(np.int64).ravel()
    attr = np.asarray(edge_attr, dtype=np.float32)
    msg = np.asarray(msg_emb, dtype=np.float32)
    We = np.asarray(We, dtype=np.float32)
    be = np.asarray(be, dtype=np.float32).ravel()
    W1 = np.asarray(W1, dtype=np.float32)
    b1 = np.asarray(b1, dtype=np.float32).ravel()
    W4 = np.asarray(W4, dtype=np.float32)
    assert dst.shape == (E,) and attr.shape == (E, A_DIM) and msg.shape == (E, D)

    order = np.argsort(dst, kind="stable")
    deg = np.bincount(dst, minlength=N).astype(np.int64)
    estart = np.zeros(N + 1, np.int64)
    np.cumsum(deg, out=estart[1:])
    half = (deg + 1) // 2  # columns per node

    # shared weight tensors; we2 replicated into all 4 32-row bands so each
    # band-matmul's stationary shares the rhs base partition (PE row tiling)
    we2 = np.zeros((128, 128), _F16)
    for u in range(4):
        we2[32 * u:32 * u + 16, 0:64] = We
        we2[32 * u + 16:32 * u + 32, 64:128] = We
    w1s = np.concatenate([W1, W1], axis=0).astype(_F16)          # [128, 64]
    w1b = np.zeros((128, 128), _F16)
    w1b[0:64, 0:64] = W1
    w1b[64:128, 64:128] = W1
    w4b = np.zeros((128, 128), _F16)
    w4b[0:64, 0:64] = W4
    w4b[64:128, 64:128] = W4
    b1h = b1.reshape(64, 1).astype(np.float32)
    b1s = np.concatenate([b1, b1]).reshape(128, 1).astype(np.float32)
    bes = np.concatenate([be, be]).reshape(128, 1).astype(np.float32)

    in_maps = []
    for c in range(NCORES):
        nlo = c * NPC
        deg_c = deg[nlo:nlo + NPC]
        half_c = half[nlo:nlo + NPC]
        estart_c = estart[nlo:nlo + NPC]

        half_g = half_c.reshape(NG, G_NODES)
        gtot = half_g.sum(axis=1)
        if not (gtot <= C).all():
            raise RuntimeError(f"group overflow: max cols {gtot.max()} > {C}")
        cum_g = np.cumsum(half_g, axis=1)            # inclusive col count
        colstart_g = cum_g - half_g                  # within-group col start
        endcol_g = cum_g - 1                         # within-group end col (-1 ok)

        # per-column arrays (real columns only)
        ncols = int(half_c.sum())
        node_of_col = np.repeat(np.arange(NPC), half_c)
        cstart_node = np.cumsum(half_c) - half_c
        k_arr = np.arange(ncols) - np.repeat(cstart_node, half_c)
        g_of_col = node_of_col // G_NODES
        colpos = (g_of_col * C
                  + colstart_g[g_of_col, node_of_col % G_NODES]
                  + k_arr).astype(np.int64)
        es = estart_c[node_of_col]
        pA = es + k_arr
        pB = es + half_c[node_of_col] + k_arr
        validB = pB < es + deg_c[node_of_col]
        eA = order[pA]
        eB = order[np.minimum(pB, E - 1)]

        # msgT packed [128, CT] fp16: rows 0:64 = A-half features, 64:128 = B
        msgT = np.zeros((128, CT), _F16)
        msgT[0:64, colpos] = msg[eA].T
        msgT[64:128, colpos[validB]] = msg[eB[validB]].T

        # attr 2-block [32, CT] then band-packed [128, NG*ACOLS] fp16
        a2 = np.zeros((32, CT), np.float32)
        a2[0:16, colpos] = attr[eA].T
        a2[16:32, colpos[validB]] = attr[eB[validB]].T
        ap_ = np.zeros((128, NG * ACOLS), _F16)
        for g in range(NG):
            blk = a2[:, g * C:(g + 1) * C].reshape(32, NCHUNK, 512)
            for q in range(NCHUNK):
                u, b = q % 4, q // 4
                ap_[32 * u:32 * u + 32,
                    g * ACOLS + 512 * b:g * ACOLS + 512 * (b + 1)] = blk[:, q]

        # gather indices: per group slots = [prev_end(=-1->ZERO)] + endcols
        gidx = np.zeros((128, NG * GS16), np.int16)
        for g in range(NG):
            slots = np.full(GSLOT, ZERO_COL, np.int64)
            ends = endcol_g[g]
            slots[1:1 + G_NODES] = np.where(ends < 0, ZERO_COL, ends)
            wrapped = slots.reshape(GS16, 16).T      # [16, GS16]; i = s*16+p
            gidx[:, g * GS16:(g + 1) * GS16] = np.tile(wrapped, (8, 1))

        in_maps.append({
            "attrp": ap_, "msgT": msgT, "we2": we2, "w1s": w1s,
            "w1b": w1b, "w4b": w4b, "b1h": b1h, "b1s": b1s, "bes": bes,
            "gidx": gidx,
        })
    return in_maps


def unpack_output(results):
    """results: list of 8 dicts with 'outp' [128, 6250] f32 -> [N, 64] f32."""
    full = np.empty((N, D), np.float32)
    for c, r in enumerate(results):
        op_ = np.asarray(r["outp"], np.float32)          # [128, 6250]
        # stacked col J = 250*blk + i holds node 500*blk + 250*beta + i
        o = op_.reshape(2, 64, HCOLS // MLP_C1, MLP_C1)   # [beta, d, blk, i]
        full[c * NPC:(c + 1) * NPC] = (
            o.transpose(2, 0, 3, 1).reshape(NPC, D))
    return full


# ---------------------------------------------------------- device program
DEBUG_OUTS = os.environ.get("GNN_DEBUG_OUTS", "0") == "1"


def build_device_program(tc, outs, ins):
    import concourse.mybir as mybir

    nc = tc.nc
    f16 = mybir.dt.float16
    f32 = mybir.dt.float32
    Alu = mybir.AluOpType
    Act = mybir.ActivationFunctionType

    outp = outs["outp"]

    with tc.sbuf_pool(name="cpool", bufs=1) as cp:
        we2_t = cp.tile_from(ins["we2"])
        w1s_t = cp.tile_from(ins["w1s"])
        w1b_t = cp.tile_from(ins["w1b"])
        w4b_t = cp.tile_from(ins["w4b"])
        b1h_t = cp.tile_from(ins["b1h"])
        b1s_t = cp.tile_from(ins["b1s"])
        bes_t = cp.tile_from(ins["bes"])
        gidx_t = cp.tile_from(ins["gidx"])
        zcol_t = cp.tile([128, 1], f16)
        nc.vector.memset(zcol_t[:], 0.0)
        node_t = cp.tile([128, NPC], f16)

        with tc.sbuf_pool(name="wpool", bufs=2) as wp, \
             tc.tile_pool(name="pspool", bufs=2, space="PSUM") as pp:
            for g in range(NG):
                msg_t = wp.tile([128, C], f16, tag="msg")
                nc.sync.dma_start(msg_t[:], ins["msgT"][:, g * C:(g + 1) * C])
                attr_t = wp.tile([128, ACOLS], f16, tag="attr")
                nc.sync.dma_start(attr_t[:],
                                  ins["attrp"][:, g * ACOLS:(g + 1) * ACOLS])
                x_t = wp.tile([128, C], f16, tag="x")
                win_t = wp.tile([128, C + 1], f32, tag="win")

                for t in range(NBANDS):
                    qlo, qhi = 4 * t, min(4 * t + 4, NCHUNK)
                    width = (qhi - qlo) * 512
                    ps_t = pp.tile([128, 2048], f32, tag="ps")
                    for u in range(qhi - qlo):
                        nc.tensor.matmul(
                            ps_t[:, 512 * u:512 * (u + 1)],
                            we2_t[32 * u:32 * (u + 1), :],
                            attr_t[32 * u:32 * (u + 1), 512 * t:512 * (t + 1)],
                            start=True, stop=True,
                            tile_position=(32 * u, 0))
                    # ACT: fp16 copy of xlin + bias (keeps DVE mul in 2x mode)
                    nc.scalar.activation(
                        x_t[:, 2048 * t:2048 * t + width],
                        ps_t[:, :width],
                        Act.Identity, bias=bes_t[:, 0:1])
                # DVE: x *= msg (all-fp16 SBUF, 2x_1P)
                nc.vector.tensor_tensor(
                    x_t[:], x_t[:], msg_t[:], op=Alu.mult)

                # running f32 column sum within the group
                nc.vector.tensor_tensor_scan(
                    win_t[:, 0:C],
                    zcol_t[:].to_broadcast([128, C]),
                    x_t[:],
                    0.0, op0=Alu.add, op1=Alu.add)
                nc.vector.memset(win_t[:, C:C + 1], 0.0)

                s1_t = wp.tile([128, GSLOT], f32, tag="s1")
                nc.gpsimd.ap_gather(
                    s1_t[:], win_t[:], gidx_t[:, g * GS16:(g + 1) * GS16],
                    channels=128, num_elems=C + 1, d=1, num_idxs=GSLOT)
                nc.vector.tensor_tensor(
                    node_t[:, g * G_NODES:(g + 1) * G_NODES],
                    s1_t[:, 1:1 + G_NODES],
                    s1_t[:, 0:G_NODES],
                    op=Alu.subtract)
                if DEBUG_OUTS and g == 0:
                    nc.sync.dma_start(outs["dbg_x"][:], x_t[:])
                    nc.sync.dma_start(outs["dbg_win"][:], win_t[:])
                    nc.sync.dma_start(outs["dbg_s1"][:], s1_t[:])
            if DEBUG_OUTS:
                nc.sync.dma_start(outs["dbg_node"][:], node_t[:])

        # ------------------------------------------------------------- MLP
        with tc.sbuf_pool(name="mpool", bufs=1) as mp, \
             tc.sbuf_pool(name="opool", bufs=3) as obp, \
             tc.tile_pool(name="mpspool", bufs=6, space="PSUM") as mpp:
            h1_t = mp.tile([128, HCOLS], f16, tag="h1")
            for k in range(NPC // MLP_C1):
                pt = mpp.tile([128, 512], f32, tag="mp")
                nc.tensor.matmul(pt[0:64, 0:MLP_C1], w1s_t[:],
                                 node_t[:, k * MLP_C1:(k + 1) * MLP_C1],
                                 start=True, stop=True)
                beta, j0 = k % 2, (k // 2) * MLP_C1
                nc.scalar.activation(
                    h1_t[64 * beta:64 * beta + 64, j0:j0 + MLP_C1],
                    pt[0:64, 0:MLP_C1], Act.Relu, bias=b1h_t[:, 0:1])

            prev = h1_t
            for layer in (2, 3):
                h_t = mp.tile([128, HCOLS], f16, tag=f"h{layer}", name=f"h{layer}")
                for c0 in range(0, HCOLS, MLP_C2):
                    w = min(MLP_C2, HCOLS - c0)
                    pt = mpp.tile([128, 512], f32, tag="mp", name="pt")
                    nc.tensor.matmul(pt[:, 0:w], w1b_t[:], prev[:, c0:c0 + w],
                                     start=True, stop=True)
                    nc.scalar.activation(h_t[:, c0:c0 + w], pt[:, 0:w],
                                         Act.Relu, bias=b1s_t[:, 0:1])
                prev = h_t

            for c0 in range(0, HCOLS, MLP_C2):
                w = min(MLP_C2, HCOLS - c0)
                pt = mpp.tile([128, 512], f32, tag="mp", name="pt")
                nc.tensor.matmul(pt[:, 0:w], w4b_t[:], prev[:, c0:c0 + w],
                                 start=True, stop=True)
                ob = obp.tile([128, MLP_C2], f32, tag="ob", name="ob")
                nc.scalar.copy(ob[:, 0:w], pt[:, 0:w])
                nc.sync.dma_start(outp[:, c0:c0 + w], ob[:, 0:w])


def build_program():
    """Build (once) the Bacc program + dram tensor APs."""
    if "nc" in _PROG:
        return _PROG["nc"]
    import concourse.bacc as bacc
    import concourse.mybir as mybir
    import concourse.tile as tile

    nc = bacc.Bacc("TRN2", debug=False, enable_asserts=False)
    f16, f32, i16 = mybir.dt.float16, mybir.dt.float32, mybir.dt.int16
    ins = {
        "attrp": nc.dram_tensor("attrp", [128, NG * ACOLS], f16,
                                kind="ExternalInput").ap(),
        "msgT": nc.dram_tensor("msgT", [128, CT], f16,
                               kind="ExternalInput").ap(),
        "we2": nc.dram_tensor("we2", [128, 128], f16, kind="ExternalInput").ap(),
        "w1s": nc.dram_tensor("w1s", [128, 64], f16, kind="ExternalInput").ap(),
        "w1b": nc.dram_tensor("w1b", [128, 128], f16, kind="ExternalInput").ap(),
        "w4b": nc.dram_tensor("w4b", [128, 128], f16, kind="ExternalInput").ap(),
        "b1h": nc.dram_tensor("b1h", [64, 1], f32, kind="ExternalInput").ap(),
        "b1s": nc.dram_tensor("b1s", [128, 1], f32, kind="ExternalInput").ap(),
        "bes": nc.dram_tensor("bes", [128, 1], f32, kind="ExternalInput").ap(),
        "gidx": nc.dram_tensor("gidx", [128, NG * GS16], i16,
                               kind="ExternalInput").ap(),
    }
    outs = {
        "outp": nc.dram_tensor("outp", [128, HCOLS], f32,
                               kind="ExternalOutput").ap(),
    }
    if DEBUG_OUTS:
        outs["dbg_x"] = nc.dram_tensor("dbg_x", [128, C], f16,
                                       kind="ExternalOutput").ap()
        outs["dbg_win"] = nc.dram_tensor("dbg_win", [128, C + 1], f32,
                                         kind="ExternalOutput").ap()
        outs["dbg_s1"] = nc.dram_tensor("dbg_s1", [128, GSLOT], f32,
                                        kind="ExternalOutput").ap()
        outs["dbg_node"] = nc.dram_tensor("dbg_node", [128, NPC], f16,
                                          kind="ExternalOutput").ap()
    with tile.TileContext(nc) as tc:
        build_device_program(tc, outs, ins)
    nc.compile()
    _PROG["nc"] = nc
    return nc


# ------------------------------------------------------------------ kernel
def kernel(edge_attr, msg_emb, edge_dst, num_nodes, We, be, W1, b1, W4,
           **_unused):
    assert int(num_nodes) == N
    in_maps = pack_inputs(edge_attr, msg_emb, edge_dst, We, be, W1, b1, W4)
    nc = build_program()

    from concourse.bass_utils import run_bass_kernel_spmd
    trace = os.environ.get("GNN_TRACE", "0") == "1"
    res = run_bass_kernel_spmd(nc, in_maps, core_ids=list(range(NCORES)),
                               trace=trace)
    kernel.last_results = res
    return unpack_output(res.results)

